# revision 4
# baseline (speedup 1.0000x reference)
"""Trainium2 Bass kernel for nn_CollapseAwareEmbedding.

Output:
  s_out [1096, 384]  - tiny embedding table (computed on every core, core 0's copy used)
  z     [1096, 1096, 128] - pair grid, sharded row-wise: core c writes rows [137c, 137c+137)

z structure: z[i,j,:] = concat(tab1[pid(i,j)//4]+b1, tab2[pid(i,j)%4]+b2) where pid is a
static function of (i,j). pid is piecewise constant on an 8x8 region grid (plus a 3-wide
diagonal band inside the hd x hd block), so each core's shard is written with ~16 large
broadcast DMAs sourced from a per-(row, col-segment) vector table V built on-device by
one-hot matmuls against the runtime tables. The one-hot selectors are per-core inputs,
keeping the SPMD program identical across cores; only the diagonal-band fixups are
conditional DMAs keyed on partition_id.
"""

import math
import numpy as np

N_CORES = 8
L = 1096
SHARD = 137          # rows per core
ZC = 128             # z channel dim
ROW_ELEMS = L * ZC   # elements per z row = 140288

# region layout: collapse(1), hd(400), mhc(400), pep(15), lv(120), lj(20), hv(120), hj(20)
BOUNDS = [0, 1, 401, 801, 816, 936, 956, 1076, 1096]
SEGS = list(zip(BOUNDS[:-1], BOUNDS[1:]))   # 8 column segments
N_COND = 6
COND_NAMES = ["mhc", "pep", "lv", "lj", "hv", "hj"]
COND_LENS = [400, 15, 120, 20, 120, 20]
HD_LEN = 400
NSEG = 11            # 8 real segments + 3 patch pseudo-segments (vals 2,0,2)
PATCH_VALS = (2, 0, 2)

D_POS = 64
MAX_LEN = 2056
TWO_PI = 2.0 * math.pi
C_HI = float(np.float32(6.28125))                    # exact in f32
C_LO = float(np.float32(TWO_PI - 6.28125))


def _region_id(i):
    """0 = collapse, 1 = hd, 2+k = conditioning region k."""
    for r in range(8):
        if BOUNDS[r] <= i < BOUNDS[r + 1]:
            return r
    raise ValueError(i)


def _inter(a, b):
    # conditioning inter-region pair id, a < b, both in [0, 6)
    return 5 + N_COND + a * (N_COND - 1) - a * (a - 1) // 2 + (b - a - 1)


def _vsel_table():
    """vsel[i, s] = pid value of (row i, col segment s) away from the hd diagonal."""
    vs = np.zeros((L, 8), np.int64)
    rid = np.array([_region_id(i) for i in range(L)])
    for i in range(L):
        ri = rid[i]
        # s = 0: collapse column
        vs[i, 0] = 0 if ri == 0 else 1
        # s = 1: hd columns
        vs[i, 1] = 1 if ri == 0 else (3 if ri == 1 else 4)
        # s >= 2: conditioning region kj = s - 2
        for s in range(2, 8):
            kj = s - 2
            if ri == 0:
                vs[i, s] = 1
            elif ri == 1:
                vs[i, s] = 4
            else:
                ki = ri - 2
                if ki == kj:
                    vs[i, s] = 5 + ki
                else:
                    vs[i, s] = _inter(min(ki, kj), max(ki, kj))
    return vs


def _onehots():
    """Per-core one-hot selector tables.

    oh1[c] : [NSEG, 32, 128]  column m -> V1 partition m -> shard row m (m in [0,128))
    oh2[c] : [32, 99]         column q = 9*s + rr -> V2 row 128+rr, segment s
    """
    vs = _vsel_table()
    oh1 = np.zeros((N_CORES, NSEG, 32, 128), np.float32)
    oh2 = np.zeros((N_CORES, 32, 99), np.float32)
    for c in range(N_CORES):
        rows = np.arange(SHARD * c, SHARD * (c + 1))
        for s in range(8):
            oh1[c, s, vs[rows[:128], s], np.arange(128)] = 1.0
            for rr in range(9):
                oh2[c, vs[rows[128 + rr], s], 9 * s + rr] = 1.0
        for t, pv in enumerate(PATCH_VALS):
            oh1[c, 8 + t, pv, :] = 1.0
            oh2[c, pv, 9 * (8 + t): 9 * (8 + t) + 9] = 1.0
    return oh1, oh2


def _coef_table():
    """[64] f32: rows 0:32 sin coefs, rows 32:64 identical (cos uses +pi/2 bias)."""
    K = np.arange(32, dtype=np.float32)
    c = np.float32(math.pi) / np.power(np.float32(MAX_LEN), (2.0 * K / 64.0).astype(np.float32))
    return np.concatenate([c, c]).astype(np.float32)


def _band_specs():
    """Per-core diagonal-band fixups for the hd x hd block.

    Returns dict c -> list of ("v1", ka, kb) / ("v2", ka, kb) full 3-wide bands over
    shard rows [ka, kb), plus ("lo", k) / ("hi", k) 2-wide edges (abs rows 1 and 400).
    """
    specs = {c: [] for c in range(N_CORES)}
    for c in range(N_CORES):
        full = [k for k in range(SHARD) if 2 <= SHARD * c + k <= HD_LEN - 1]
        v1 = [k for k in full if k < 128]
        v2 = [k for k in full if k >= 128]
        if v1:
            specs[c].append(("v1", v1[0], v1[-1] + 1))
        if v2:
            specs[c].append(("v2", v2[0], v2[-1] + 1))
        for k in range(SHARD):
            if SHARD * c + k == 1:
                specs[c].append(("lo", k, k + 1))
            if SHARD * c + k == HD_LEN:
                specs[c].append(("hi", k, k + 1))
    return specs


_PROG = None


def _build_program():
    import concourse.bacc as bacc
    import concourse.tile as tile
    import concourse.bass as bass
    import concourse.mybir as mybir
    from concourse.masks import make_identity

    F32 = mybir.dt.float32
    I32 = mybir.dt.int32
    Sin = mybir.ActivationFunctionType.Sin
    SUB = mybir.AluOpType.subtract
    ADD = mybir.AluOpType.add
    MULT = mybir.AluOpType.mult

    nc = bacc.Bacc("TRN2", target_bir_lowering=False, debug=False, num_devices=N_CORES)

    # ---- inputs ----
    inp = {}
    def din(name, shape, dt=F32):
        inp[name] = nc.dram_tensor(name, shape, dt, kind="ExternalInput")
        return inp[name]

    hd_in = din("hd", [HD_LEN, 21])
    mask_in = din("mask", [HD_LEN])
    cond_ins = {}
    idx_ins = {}
    for nm, ln in zip(COND_NAMES, COND_LENS):
        cond_ins[nm] = din(nm, [ln, 21])
        idx_ins[nm] = din(nm + "_idx32", [ln], I32)
    seqW_in = din("seq_W", [22, 384])
    seqb_in = din("seq_b", [384])
    posW_in = din("pos_W", [64, 384])
    posb_in = din("pos_b", [384])
    tab1_in = din("tab1", [8, 64])
    b1_in = din("b1", [64])
    tab2_in = din("tab2", [4, 64])
    b2_in = din("b2", [64])
    ct_in = din("collapse_token", [1, 384])
    cw_in = din("collapse_weight", [1])
    rw_in = din("region_w", [6, 2])
    oh1_in = din("oh1", [NSEG, 32, 128])
    oh2_in = din("oh2", [32, 99])
    coef_in = din("coef64", [64])

    z_t = nc.dram_tensor("z_shard", [SHARD, L, ZC], F32, kind="ExternalOutput")
    s_t = nc.dram_tensor("s_out", [L, 384], F32, kind="ExternalOutput")
    z_h = z_t  # tensor handle for custom APs

    band = _band_specs()

    with tile.TileContext(nc) as tc:
        with tc.tile_pool(name="consts", bufs=1) as cpool, \
             tc.tile_pool(name="work", bufs=3) as wpool, \
             tc.tile_pool(name="ps_v", bufs=2, space="PSUM") as ps_v, \
             tc.tile_pool(name="ps_x", bufs=1, space="PSUM") as ps_x, \
             tc.tile_pool(name="ps_t", bufs=2, space="PSUM") as ps_t, \
             tc.tile_pool(name="ps_s", bufs=2, space="PSUM") as ps_s:

            # ---------- LUT [32, 128]: lut[v] = [tab1[v//4]+b1 | tab2[v%4]+b2] ----------
            t1r = cpool.tile([32, 64], F32, tag="t1r")
            nc.sync.dma_start(t1r[:], tab1_in[:].unsqueeze(1).broadcast_to([8, 4, 64]))
            t2r = cpool.tile([32, 64], F32, tag="t2r")
            nc.sync.dma_start(t2r[:], tab2_in[:].unsqueeze(0).broadcast_to([8, 4, 64]))
            b1r = cpool.tile([32, 64], F32, tag="b1r")
            nc.sync.dma_start(b1r[:], b1_in[:].unsqueeze(0).broadcast_to([32, 64]))
            b2r = cpool.tile([32, 64], F32, tag="b2r")
            nc.sync.dma_start(b2r[:], b2_in[:].unsqueeze(0).broadcast_to([32, 64]))
            lut = cpool.tile([32, 128], F32, tag="lut")
            nc.vector.tensor_tensor(lut[:, 0:64], t1r[:], b1r[:], ADD)
            nc.vector.tensor_tensor(lut[:, 64:128], t2r[:], b2r[:], ADD)

            # ---------- one-hot tables ----------
            oh1_t = cpool.tile([32, NSEG * 128], F32, tag="oh1t")
            nc.sync.dma_start(oh1_t[:], oh1_in[:].transpose([1, 0, 2]))
            oh2_t = cpool.tile([32, 99], F32, tag="oh2t")
            nc.sync.dma_start(oh2_t[:], oh2_in[:])

            # ---------- V tables ----------
            V1 = cpool.tile([128, NSEG * 128], F32, tag="V1")
            for s in range(NSEG):
                ps = ps_v.tile([128, 128], F32)
                nc.tensor.matmul(ps[:], oh1_t[:, s * 128:(s + 1) * 128], lut[:],
                                 start=True, stop=True)
                nc.vector.tensor_copy(V1[:, s * 128:(s + 1) * 128], ps[:])
            V2 = cpool.tile([99, 128], F32, tag="V2")
            ps2 = ps_x.tile([99, 128], F32, tag="ps2")
            nc.tensor.matmul(ps2[:], oh2_t[:], lut[:], start=True, stop=True)
            nc.vector.tensor_copy(V2[:], ps2[:])
            # contiguous 3-vector diagonal patch for shard rows 128..136:
            # Vp2[rr, t*128+c] = lut[PATCH_VALS[t], c]
            Vp2 = cpool.tile([9, 3 * 128], F32, tag="Vp2")
            psP = ps_x.tile([9, 3 * 128], F32, tag="psP")
            for t in range(3):
                nc.tensor.matmul(psP[0:9, t * 128:(t + 1) * 128],
                                 oh2_t[:, 9 * (8 + t): 9 * (8 + t) + 9], lut[:],
                                 start=True, stop=True)
            nc.vector.tensor_copy(Vp2[:], psP[:])

            # ---------- bulk z DMAs: 8 segments x (rows 0..127 | rows 128..136) ----------
            for si, (j0, j1) in enumerate(SEGS):
                w = j1 - j0
                eng = nc.sync if si % 2 == 0 else nc.scalar
                src1 = V1[:, si * 128:(si + 1) * 128].unsqueeze(1).broadcast_to([128, w, ZC])
                eng.dma_start(z_t[0:128, j0:j1, :], src1)
                src2 = V2[si * 9: si * 9 + 9, :].unsqueeze(1).broadcast_to([9, w, ZC])
                eng.dma_start(z_t[128:SHARD, j0:j1, :], src2)

            # ---------- conditional diagonal-band fixups ----------
            pid_sp = nc.sync.partition_id()
            eqs = {c: (pid_sp == c) for c in range(N_CORES) if band[c]}
            for c, items in band.items():
                if not items:
                    continue
                eq = eqs[c]
                for kind, ka, kb in items:
                    n = kb - ka
                    if kind in ("v1", "v2"):
                        off = ka * ROW_ELEMS + (SHARD * c + ka - 1) * ZC
                        dst = bass.AP(tensor=z_h, offset=off,
                                      ap=[[ROW_ELEMS + ZC, n], [1, 3 * ZC]])
                        if kind == "v1":
                            src = V1[ka:kb, 8 * 128: 11 * 128]
                        else:
                            rr0 = ka - 128
                            src = Vp2[rr0:rr0 + n, :]
                        dst = nc.ap_or_oob(dst, eq)
                        nc.sync.dma_start(dst, src, bounds_check="skip_entire_dma")
                    elif kind == "lo":   # abs row 1: cols (1,2) = vals (0,2) = patch segs 9,10
                        dst = nc.ap_or_oob(z_t[ka:kb, 1:3, :], eq)
                        nc.sync.dma_start(dst, V1[ka:kb, 9 * 128: 11 * 128],
                                          bounds_check="skip_entire_dma")
                    else:                # abs row 400: cols (399,400) = vals (2,0) = segs 8,9
                        dst = nc.ap_or_oob(z_t[ka:kb, 399:401, :], eq)
                        nc.sync.dma_start(dst, V1[ka:kb, 8 * 128: 10 * 128],
                                          bounds_check="skip_entire_dma")

            # ---------- s_out ----------
            ident = cpool.tile([128, 128], F32, tag="ident")
            make_identity(nc, ident[:])

            seqWb = cpool.tile([23, 384], F32, tag="seqWb")   # row 0 = seq_b, rows 1:23 = seq_W
            nc.scalar.dma_start(seqWb[0:1, :], seqb_in[:].unsqueeze(0))
            nc.scalar.dma_start(seqWb[1:23, :], seqW_in[:])
            posW = cpool.tile([64, 384], F32, tag="posW")
            nc.scalar.dma_start(posW[:], posW_in[:])
            posb = cpool.tile([1, 384], F32, tag="posb")
            nc.scalar.dma_start(posb[:], posb_in[:].unsqueeze(0))
            coef_t = cpool.tile([64, 1], F32, tag="coef")
            nc.scalar.dma_start(coef_t[:], coef_in[:].rearrange("(p one) -> p one", one=1))
            qoff = cpool.tile([64, 1], F32, tag="qoff")
            nc.vector.memset(qoff[0:32], 0.0)
            nc.vector.memset(qoff[32:64], 0.25)
            sbias = cpool.tile([64, 1], F32, tag="sbias")
            nc.vector.memset(sbias[0:32], 0.0)
            nc.vector.memset(sbias[32:64], math.pi / 2.0)

            # region weights, scaled rhs tables
            rhs_k = {}
            posW_k = {}
            for k in range(N_COND):
                w0 = cpool.tile([128, 1], F32, tag=f"w0_{k}")
                nc.scalar.dma_start(
                    w0[:], rw_in[k:k + 1, 0:1].broadcast_to([128, 1]))
                w1 = cpool.tile([128, 1], F32, tag=f"w1_{k}")
                nc.scalar.dma_start(
                    w1[:], rw_in[k:k + 1, 1:2].broadcast_to([128, 1]))
                rk = cpool.tile([23, 384], F32, tag=f"rhs_{k}")
                nc.vector.tensor_scalar_mul(rk[:], seqWb[:], w0[0:23, 0:1])
                tmp = wpool.tile([1, 384], F32, tag="rtmp")
                nc.vector.tensor_scalar_mul(tmp[:], posb[:], w1[0:1, 0:1])
                nc.vector.tensor_tensor(rk[0:1, :], rk[0:1, :], tmp[:], ADD)
                rhs_k[k] = rk
                pk = cpool.tile([64, 384], F32, tag=f"posWk_{k}")
                nc.vector.tensor_scalar_mul(pk[:], posW[:], w1[0:64, 0:1])
                posW_k[k] = pk

            # collapse row
            ct = cpool.tile([1, 384], F32, tag="ct")
            nc.scalar.dma_start(ct[:], ct_in[:])
            cw = cpool.tile([1, 1], F32, tag="cw")
            nc.scalar.dma_start(cw[:], cw_in[:].unsqueeze(0))
            s0 = cpool.tile([1, 384], F32, tag="s0")
            nc.vector.tensor_scalar_mul(s0[:], ct[:], cw[0:1, 0:1])
            nc.scalar.dma_start(s_t[0:1, :], s0[:])

            # per-region chunks
            regions = [("hd", HD_LEN, 1, None)] + [
                (nm, ln, BOUNDS[2 + k], k) for k, (nm, ln) in enumerate(zip(COND_NAMES, COND_LENS))
            ]
            for nm, ln, base, k in regions:
                is_hd = k is None
                x_in = hd_in if is_hd else cond_ins[nm]
                if not is_hd:
                    idxi = cpool.tile([1, ln], I32, tag=f"idxi_{nm}")
                    nc.scalar.dma_start(idxi[:], idx_ins[nm][:].unsqueeze(0))
                    idxf = cpool.tile([1, ln], F32, tag=f"idxf_{nm}")
                    nc.vector.tensor_copy(idxf[:], idxi[:])
                for cs in range(0, ln, 128):
                    n = min(128, ln - cs)
                    aa = wpool.tile([128, 23], F32, tag="aa")
                    nc.vector.memset(aa[0:n, 0:1], 1.0)
                    if is_hd:
                        nc.scalar.dma_start(
                            aa[0:n, 1:2],
                            mask_in[cs:cs + n].rearrange("(p one) -> p one", one=1))
                    else:
                        nc.vector.memset(aa[0:n, 1:2], 0.0)
                    nc.scalar.dma_start(aa[0:n, 2:23], x_in[cs:cs + n, :])
                    psT = ps_t.tile([23, 128], F32)
                    nc.tensor.transpose(psT[0:23, 0:n], aa[0:n, 0:23], ident[0:n, 0:n])
                    aaT = wpool.tile([23, 128], F32, tag="aaT")
                    nc.vector.tensor_copy(aaT[0:23, 0:n], psT[0:23, 0:n])
                    psS = ps_s.tile([128, 384], F32)
                    nc.tensor.matmul(psS[0:n, :], aaT[0:23, 0:n],
                                     seqWb[:] if is_hd else rhs_k[k][:],
                                     start=True, stop=is_hd)
                    if not is_hd:
                        idxb = wpool.tile([64, 128], F32, tag="idxb")
                        nc.gpsimd.partition_broadcast(idxb[:, 0:n], idxf[0:1, cs:cs + n])
                        ang = wpool.tile([64, 128], F32, tag="ang")
                        nc.vector.tensor_scalar_mul(ang[:, 0:n], idxb[:, 0:n], coef_t[:, 0:1])
                        q = wpool.tile([64, 128], F32, tag="q")
                        nc.vector.tensor_scalar(q[:, 0:n], ang[:, 0:n],
                                                1.0 / TWO_PI, qoff[:, 0:1], MULT, ADD)
                        qi = wpool.tile([64, 128], I32, tag="qi")
                        nc.vector.tensor_copy(qi[:, 0:n], q[:, 0:n])
                        qf = wpool.tile([64, 128], F32, tag="qf")
                        nc.vector.tensor_copy(qf[:, 0:n], qi[:, 0:n])
                        tt = wpool.tile([64, 128], F32, tag="tt")
                        nc.vector.tensor_scalar_mul(tt[:, 0:n], qf[:, 0:n], C_HI)
                        rr = wpool.tile([64, 128], F32, tag="rr")
                        nc.vector.tensor_tensor(rr[:, 0:n], ang[:, 0:n], tt[:, 0:n], SUB)
                        nc.vector.tensor_scalar_mul(tt[:, 0:n], qf[:, 0:n], C_LO)
                        nc.vector.tensor_tensor(rr[:, 0:n], rr[:, 0:n], tt[:, 0:n], SUB)
                        onedT = wpool.tile([64, 128], F32, tag="onedT")
                        nc.scalar.activation(onedT[:, 0:n], rr[:, 0:n], Sin,
                                             bias=sbias[:, 0:1])
                        nc.tensor.matmul(psS[0:n, :], onedT[:, 0:n], posW_k[k][:],
                                         start=False, stop=True)
                    sres = wpool.tile([128, 384], F32, tag="sres")
                    nc.vector.tensor_copy(sres[0:n, :], psS[0:n, :])
                    nc.scalar.dma_start(s_t[base + cs: base + cs + n, :], sres[0:n, :])

    nc.compile()
    return nc


def _get_prog():
    global _PROG
    if _PROG is None:
        _PROG = _build_program()
    return _PROG


_OH1, _OH2 = None, None


def kernel(**inputs):
    global _OH1, _OH2
    import os
    from concourse.bass_utils import run_bass_kernel_spmd

    nc = _get_prog()
    if _OH1 is None:
        _OH1, _OH2 = _onehots()
    coef = _coef_table()

    f32 = np.float32
    common = {
        "hd": np.ascontiguousarray(inputs["hd"], f32),
        "mask": np.ascontiguousarray(inputs["mask"], f32),
        "seq_W": np.ascontiguousarray(inputs["seq_W"], f32),
        "seq_b": np.ascontiguousarray(inputs["seq_b"], f32),
        "pos_W": np.ascontiguousarray(inputs["pos_W"], f32),
        "pos_b": np.ascontiguousarray(inputs["pos_b"], f32),
        "tab1": np.ascontiguousarray(inputs["tab1"], f32),
        "b1": np.ascontiguousarray(inputs["b1"], f32),
        "tab2": np.ascontiguousarray(inputs["tab2"], f32),
        "b2": np.ascontiguousarray(inputs["b2"], f32),
        "collapse_token": np.ascontiguousarray(inputs["collapse_token"], f32),
        "collapse_weight": np.ascontiguousarray(inputs["collapse_weight"], f32),
        "region_w": np.ascontiguousarray(inputs["region_w"], f32),
        "coef64": coef,
    }
    for nm in COND_NAMES:
        common[nm] = np.ascontiguousarray(inputs[nm], f32)
        common[nm + "_idx32"] = np.ascontiguousarray(inputs[nm + "_idx"]).astype(np.int32)

    in_maps = [dict(common, oh1=_OH1[c], oh2=_OH2[c]) for c in range(N_CORES)]

    trace = bool(int(os.environ.get("BASS_KERNEL_TRACE", "0")))
    last_exc = None
    for _attempt in range(3):
        try:
            res = run_bass_kernel_spmd(nc, in_maps, core_ids=list(range(N_CORES)),
                                       trace=trace)
            break
        except Exception as e:   # transient LoadExecutable failures seen on axon
            last_exc = e
    else:
        raise last_exc

    kernel.last_results = res
    z = np.concatenate([res.results[c]["z_shard"] for c in range(N_CORES)], axis=0)
    s_out = res.results[0]["s_out"]
    return s_out, z


# revision 6
# speedup vs baseline: 1.1189x; 1.1189x over previous
"""Trainium2 Bass kernel for nn_CollapseAwareEmbedding.

Output:
  s_out [1096, 384]  - tiny embedding table (computed on every core, core 0's copy used)
  z     [1096, 1096, 128] - pair grid, sharded row-wise: core c writes rows [137c, 137c+137)

z structure: z[i,j,:] = concat(tab1[pid(i,j)//4]+b1, tab2[pid(i,j)%4]+b2) where pid is a
static function of (i,j). pid is piecewise constant on an 8x8 region grid (plus a 3-wide
diagonal band inside the hd x hd block), so each core's shard is written with ~16 large
broadcast DMAs sourced from a per-(row, col-segment) vector table V built on-device by
one-hot matmuls against the runtime tables. The one-hot selectors are per-core inputs,
keeping the SPMD program identical across cores; only the diagonal-band fixups are
conditional DMAs keyed on partition_id.
"""

import math
import numpy as np

N_CORES = 8
L = 1096
SHARD = 137          # rows per core
ZC = 128             # z channel dim
ROW_ELEMS = L * ZC   # elements per z row = 140288

# region layout: collapse(1), hd(400), mhc(400), pep(15), lv(120), lj(20), hv(120), hj(20)
BOUNDS = [0, 1, 401, 801, 816, 936, 956, 1076, 1096]
SEGS = list(zip(BOUNDS[:-1], BOUNDS[1:]))   # 8 column segments
N_COND = 6
COND_NAMES = ["mhc", "pep", "lv", "lj", "hv", "hj"]
COND_LENS = [400, 15, 120, 20, 120, 20]
HD_LEN = 400
NSEG = 11            # 8 real segments + 3 patch pseudo-segments (vals 2,0,2)
PATCH_VALS = (2, 0, 2)

D_POS = 64
MAX_LEN = 2056
TWO_PI = 2.0 * math.pi
C_HI = float(np.float32(6.28125))                    # exact in f32
C_LO = float(np.float32(TWO_PI - 6.28125))


def _region_id(i):
    """0 = collapse, 1 = hd, 2+k = conditioning region k."""
    for r in range(8):
        if BOUNDS[r] <= i < BOUNDS[r + 1]:
            return r
    raise ValueError(i)


def _inter(a, b):
    # conditioning inter-region pair id, a < b, both in [0, 6)
    return 5 + N_COND + a * (N_COND - 1) - a * (a - 1) // 2 + (b - a - 1)


def _vsel_table():
    """vsel[i, s] = pid value of (row i, col segment s) away from the hd diagonal."""
    vs = np.zeros((L, 8), np.int64)
    rid = np.array([_region_id(i) for i in range(L)])
    for i in range(L):
        ri = rid[i]
        # s = 0: collapse column
        vs[i, 0] = 0 if ri == 0 else 1
        # s = 1: hd columns
        vs[i, 1] = 1 if ri == 0 else (3 if ri == 1 else 4)
        # s >= 2: conditioning region kj = s - 2
        for s in range(2, 8):
            kj = s - 2
            if ri == 0:
                vs[i, s] = 1
            elif ri == 1:
                vs[i, s] = 4
            else:
                ki = ri - 2
                if ki == kj:
                    vs[i, s] = 5 + ki
                else:
                    vs[i, s] = _inter(min(ki, kj), max(ki, kj))
    return vs


def _onehots():
    """Per-core one-hot selector tables.

    oh1[c] : [NSEG, 32, 128]  column m -> V1 partition m -> shard row m (m in [0,128))
    oh2[c] : [32, 99]         column q = 9*s + rr -> V2 row 128+rr, segment s
    """
    vs = _vsel_table()
    oh1 = np.zeros((N_CORES, NSEG, 32, 128), np.float32)
    oh2 = np.zeros((N_CORES, 32, 99), np.float32)
    for c in range(N_CORES):
        rows = np.arange(SHARD * c, SHARD * (c + 1))
        for s in range(8):
            oh1[c, s, vs[rows[:128], s], np.arange(128)] = 1.0
            for rr in range(9):
                oh2[c, vs[rows[128 + rr], s], 9 * s + rr] = 1.0
        for t, pv in enumerate(PATCH_VALS):
            oh1[c, 8 + t, pv, :] = 1.0
            oh2[c, pv, 9 * (8 + t): 9 * (8 + t) + 9] = 1.0
    return oh1, oh2


def _coef_table():
    """[64] f32: rows 0:32 sin coefs, rows 32:64 identical (cos uses +pi/2 bias)."""
    K = np.arange(32, dtype=np.float32)
    c = np.float32(math.pi) / np.power(np.float32(MAX_LEN), (2.0 * K / 64.0).astype(np.float32))
    return np.concatenate([c, c]).astype(np.float32)


def _band_specs():
    """Per-core diagonal-band fixups for the hd x hd block.

    Returns dict c -> list of ("v1", ka, kb) / ("v2", ka, kb) full 3-wide bands over
    shard rows [ka, kb), plus ("lo", k) / ("hi", k) 2-wide edges (abs rows 1 and 400).
    """
    specs = {c: [] for c in range(N_CORES)}
    for c in range(N_CORES):
        full = [k for k in range(SHARD) if 2 <= SHARD * c + k <= HD_LEN - 1]
        v1 = [k for k in full if k < 128]
        v2 = [k for k in full if k >= 128]
        if v1:
            specs[c].append(("v1", v1[0], v1[-1] + 1))
        if v2:
            specs[c].append(("v2", v2[0], v2[-1] + 1))
        for k in range(SHARD):
            if SHARD * c + k == 1:
                specs[c].append(("lo", k, k + 1))
            if SHARD * c + k == HD_LEN:
                specs[c].append(("hi", k, k + 1))
    return specs


_PROG = None


def _build_program():
    import concourse.bacc as bacc
    import concourse.tile as tile
    import concourse.bass as bass
    import concourse.mybir as mybir
    from concourse.masks import make_identity

    F32 = mybir.dt.float32
    I32 = mybir.dt.int32
    Sin = mybir.ActivationFunctionType.Sin
    SUB = mybir.AluOpType.subtract
    ADD = mybir.AluOpType.add
    MULT = mybir.AluOpType.mult

    nc = bacc.Bacc("TRN2", target_bir_lowering=False, debug=False, num_devices=N_CORES)

    # ---- inputs ----
    inp = {}
    def din(name, shape, dt=F32):
        inp[name] = nc.dram_tensor(name, shape, dt, kind="ExternalInput")
        return inp[name]

    hd_in = din("hd", [HD_LEN, 21])
    mask_in = din("mask", [HD_LEN])
    cond_ins = {}
    idx_ins = {}
    for nm, ln in zip(COND_NAMES, COND_LENS):
        cond_ins[nm] = din(nm, [ln, 21])
        idx_ins[nm] = din(nm + "_idx32", [ln], I32)
    seqW_in = din("seq_W", [22, 384])
    seqb_in = din("seq_b", [384])
    posW_in = din("pos_W", [64, 384])
    posb_in = din("pos_b", [384])
    tab1_in = din("tab1", [8, 64])
    b1_in = din("b1", [64])
    tab2_in = din("tab2", [4, 64])
    b2_in = din("b2", [64])
    ct_in = din("collapse_token", [1, 384])
    cw_in = din("collapse_weight", [1])
    rw_in = din("region_w", [6, 2])
    oh1_in = din("oh1", [NSEG, 32, 128])
    oh2_in = din("oh2", [32, 99])
    coef_in = din("coef64", [64])

    z_t = nc.dram_tensor("z_shard", [SHARD, L, ZC], F32, kind="ExternalOutput")
    s_t = nc.dram_tensor("s_out", [L, 384], F32, kind="ExternalOutput")
    z_h = z_t  # tensor handle for custom APs

    band = _band_specs()

    REP = 8                      # vector copies per segment in the wide tables
    SEG_ORDER = [1, 2, 4, 6, 3, 5, 7, 0]          # big segments first
    SEG_REP = {0: 1, 1: 8, 2: 8, 3: 5, 4: 8, 5: 4, 6: 8, 7: 4}   # rep | width
    # ring assignment balanced by bytes: chunk1 on opposite rings for hd/mhc etc.
    RING1 = {1: 0, 2: 1, 4: 0, 6: 1, 3: 1, 5: 0, 7: 1, 0: 0}      # chunk1: 0=sync 1=scalar
    RING2 = {s: 1 - r for s, r in RING1.items()}                   # chunk2: opposite ring

    with tile.TileContext(nc) as tc:
        with tc.tile_pool(name="consts", bufs=1) as cpool, \
             tc.tile_pool(name="work", bufs=3) as wpool, \
             tc.tile_pool(name="ps_v", bufs=3, space="PSUM") as ps_v, \
             tc.tile_pool(name="ps_t", bufs=2, space="PSUM") as ps_t:

            # ---------- LUT [32, 128]: lut[v] = [tab1[v//4]+b1 | tab2[v%4]+b2] ----------
            t1r = cpool.tile([32, 64], F32, tag="t1r")
            nc.sync.dma_start(t1r[:], tab1_in[:].unsqueeze(1).broadcast_to([8, 4, 64]))
            t2r = cpool.tile([32, 64], F32, tag="t2r")
            nc.sync.dma_start(t2r[:], tab2_in[:].unsqueeze(0).broadcast_to([8, 4, 64]))
            b1r = cpool.tile([32, 64], F32, tag="b1r")
            nc.sync.dma_start(b1r[:], b1_in[:].unsqueeze(0).broadcast_to([32, 64]))
            b2r = cpool.tile([32, 64], F32, tag="b2r")
            nc.sync.dma_start(b2r[:], b2_in[:].unsqueeze(0).broadcast_to([32, 64]))
            lut = cpool.tile([32, 128], F32, tag="lut")
            nc.vector.tensor_tensor(lut[:, 0:64], t1r[:], b1r[:], ADD)
            nc.vector.tensor_tensor(lut[:, 64:128], t2r[:], b2r[:], ADD)

            # ---------- one-hot tables ----------
            oh1_t = cpool.tile([32, NSEG * 128], F32, tag="oh1t")
            nc.sync.dma_start(oh1_t[:], oh1_in[:].transpose([1, 0, 2]))
            oh2_t = cpool.tile([32, 99], F32, tag="oh2t")
            nc.sync.dma_start(oh2_t[:], oh2_in[:])

            # lut replicated REP x along free dim -> descriptors of REP*512 B
            lutw = cpool.tile([32, REP * 128], F32, tag="lutw")
            nc.vector.tensor_copy(
                lutw[:], lut[:].unsqueeze(1).broadcast_to([32, REP, 128]))

            # ---------- V tables (per segment, wide) + bulk z DMAs ----------
            engs = [nc.sync, nc.scalar]
            Vp1 = cpool.tile([128, 3 * 128], F32, tag="Vp1")
            Vp2 = cpool.tile([9, 3 * 128], F32, tag="Vp2")

            for si in SEG_ORDER:
                j0, j1 = SEGS[si]
                w = j1 - j0
                rep = SEG_REP[si]
                vw = cpool.tile([128, REP * 128], F32, tag=f"vw{si}")
                psv = ps_v.tile([128, REP * 128], F32, tag="v")
                for h in range(REP * 128 // 512):
                    nc.tensor.matmul(psv[:, h * 512:(h + 1) * 512],
                                     oh1_t[:, si * 128:(si + 1) * 128],
                                     lutw[:, h * 512:(h + 1) * 512],
                                     start=True, stop=True)
                nc.vector.tensor_copy(vw[:], psv[:])
                src1 = vw[:, 0:rep * 128].unsqueeze(1).broadcast_to(
                    [128, w // rep, rep * 128])
                engs[RING1[si]].dma_start(z_t[0:128, j0:j1, :], src1)

                v2 = cpool.tile([9, REP * 128], F32, tag=f"v2_{si}")
                psv2 = ps_v.tile([9, REP * 128], F32, tag="v")
                for h in range(REP * 128 // 512):
                    nc.tensor.matmul(psv2[:, h * 512:(h + 1) * 512],
                                     oh2_t[:, si * 9: si * 9 + 9],
                                     lutw[:, h * 512:(h + 1) * 512],
                                     start=True, stop=True)
                nc.vector.tensor_copy(v2[:], psv2[:])
                src2 = v2[:, 0:rep * 128].unsqueeze(1).broadcast_to(
                    [9, w // rep, rep * 128])
                engs[RING2[si]].dma_start(z_t[128:SHARD, j0:j1, :], src2)

            # contiguous 3-vector diagonal patches (vals 2,0,2)
            psP = ps_v.tile([128, 3 * 128], F32, tag="v")
            for t in range(3):
                nc.tensor.matmul(psP[0:128, t * 128:(t + 1) * 128],
                                 oh1_t[:, (8 + t) * 128:(9 + t) * 128], lut[:],
                                 start=True, stop=True)
            nc.vector.tensor_copy(Vp1[:], psP[:])
            psP2 = ps_v.tile([9, 3 * 128], F32, tag="v")
            for t in range(3):
                nc.tensor.matmul(psP2[0:9, t * 128:(t + 1) * 128],
                                 oh2_t[:, 9 * (8 + t): 9 * (8 + t) + 9], lut[:],
                                 start=True, stop=True)
            nc.vector.tensor_copy(Vp2[:], psP2[:])

            # ---------- conditional diagonal-band fixups ----------
            pid_sp = nc.sync.partition_id()
            eqs = {c: (pid_sp == c) for c in range(N_CORES) if band[c]}
            for c, items in band.items():
                if not items:
                    continue
                eq = eqs[c]
                for kind, ka, kb in items:
                    n = kb - ka
                    if kind in ("v1", "v2"):
                        off = ka * ROW_ELEMS + (SHARD * c + ka - 1) * ZC
                        dst = bass.AP(tensor=z_h, offset=off,
                                      ap=[[ROW_ELEMS + ZC, n], [1, 3 * ZC]])
                        if kind == "v1":
                            src = Vp1[ka:kb, 0:384]
                        else:
                            rr0 = ka - 128
                            src = Vp2[rr0:rr0 + n, :]
                        dst = nc.ap_or_oob(dst, eq)
                        nc.sync.dma_start(dst, src, bounds_check="skip_entire_dma")
                    elif kind == "lo":   # abs row 1: cols (1,2) = vals (0,2) = patch segs 9,10
                        dst = nc.ap_or_oob(z_t[ka:kb, 1:3, :], eq)
                        nc.sync.dma_start(dst, Vp1[ka:kb, 128:384],
                                          bounds_check="skip_entire_dma")
                    else:                # abs row 400: cols (399,400) = vals (2,0) = segs 8,9
                        dst = nc.ap_or_oob(z_t[ka:kb, 399:401, :], eq)
                        nc.sync.dma_start(dst, Vp1[ka:kb, 0:256],
                                          bounds_check="skip_entire_dma")

            # ---------- s_out ----------
            ident = cpool.tile([128, 128], F32, tag="ident")
            make_identity(nc, ident[:])

            seqWb = cpool.tile([23, 384], F32, tag="seqWb")   # row 0 = seq_b, rows 1:23 = seq_W
            nc.scalar.dma_start(seqWb[0:1, :], seqb_in[:].unsqueeze(0))
            nc.scalar.dma_start(seqWb[1:23, :], seqW_in[:])
            posW = cpool.tile([64, 384], F32, tag="posW")
            nc.scalar.dma_start(posW[:], posW_in[:])
            posb = cpool.tile([1, 384], F32, tag="posb")
            nc.scalar.dma_start(posb[:], posb_in[:].unsqueeze(0))
            coef_t = cpool.tile([64, 1], F32, tag="coef")
            nc.scalar.dma_start(coef_t[:], coef_in[:].rearrange("(p one) -> p one", one=1))
            qoff = cpool.tile([64, 1], F32, tag="qoff")
            nc.vector.memset(qoff[0:32], 0.0)
            nc.vector.memset(qoff[32:64], 0.25)
            sbias = cpool.tile([64, 1], F32, tag="sbias")
            nc.vector.memset(sbias[0:32], 0.0)
            nc.vector.memset(sbias[32:64], math.pi / 2.0)

            # region weights, scaled rhs tables
            rhs_k = {}
            posW_k = {}
            for k in range(N_COND):
                w0 = cpool.tile([128, 1], F32, tag=f"w0_{k}")
                nc.scalar.dma_start(
                    w0[:], rw_in[k:k + 1, 0:1].broadcast_to([128, 1]))
                w1 = cpool.tile([128, 1], F32, tag=f"w1_{k}")
                nc.scalar.dma_start(
                    w1[:], rw_in[k:k + 1, 1:2].broadcast_to([128, 1]))
                rk = cpool.tile([23, 384], F32, tag=f"rhs_{k}")
                nc.vector.tensor_scalar_mul(rk[:], seqWb[:], w0[0:23, 0:1])
                tmp = wpool.tile([1, 384], F32, tag="rtmp")
                nc.vector.tensor_scalar_mul(tmp[:], posb[:], w1[0:1, 0:1])
                nc.vector.tensor_tensor(rk[0:1, :], rk[0:1, :], tmp[:], ADD)
                rhs_k[k] = rk
                pk = cpool.tile([64, 384], F32, tag=f"posWk_{k}")
                nc.vector.tensor_scalar_mul(pk[:], posW[:], w1[0:64, 0:1])
                posW_k[k] = pk

            # collapse row
            ct = cpool.tile([1, 384], F32, tag="ct")
            nc.scalar.dma_start(ct[:], ct_in[:])
            cw = cpool.tile([1, 1], F32, tag="cw")
            nc.scalar.dma_start(cw[:], cw_in[:].unsqueeze(0))
            s0 = cpool.tile([1, 384], F32, tag="s0")
            nc.vector.tensor_scalar_mul(s0[:], ct[:], cw[0:1, 0:1])
            nc.scalar.dma_start(s_t[0:1, :], s0[:])

            # per-region chunks
            regions = [("hd", HD_LEN, 1, None)] + [
                (nm, ln, BOUNDS[2 + k], k) for k, (nm, ln) in enumerate(zip(COND_NAMES, COND_LENS))
            ]
            for nm, ln, base, k in regions:
                is_hd = k is None
                x_in = hd_in if is_hd else cond_ins[nm]
                if not is_hd:
                    idxi = cpool.tile([1, ln], I32, tag=f"idxi_{nm}")
                    nc.scalar.dma_start(idxi[:], idx_ins[nm][:].unsqueeze(0))
                    idxf = cpool.tile([1, ln], F32, tag=f"idxf_{nm}")
                    nc.vector.tensor_copy(idxf[:], idxi[:])
                for cs in range(0, ln, 128):
                    n = min(128, ln - cs)
                    aa = wpool.tile([128, 23], F32, tag="aa")
                    nc.vector.memset(aa[0:n, 0:1], 1.0)
                    if is_hd:
                        nc.scalar.dma_start(
                            aa[0:n, 1:2],
                            mask_in[cs:cs + n].rearrange("(p one) -> p one", one=1))
                    else:
                        nc.vector.memset(aa[0:n, 1:2], 0.0)
                    nc.scalar.dma_start(aa[0:n, 2:23], x_in[cs:cs + n, :])
                    psT = ps_t.tile([23, 128], F32)
                    nc.tensor.transpose(psT[0:23, 0:n], aa[0:n, 0:23], ident[0:n, 0:n])
                    aaT = wpool.tile([23, 128], F32, tag="aaT")
                    nc.vector.tensor_copy(aaT[0:23, 0:n], psT[0:23, 0:n])
                    psS = ps_v.tile([128, 384], F32, tag="v")
                    nc.tensor.matmul(psS[0:n, :], aaT[0:23, 0:n],
                                     seqWb[:] if is_hd else rhs_k[k][:],
                                     start=True, stop=is_hd)
                    if not is_hd:
                        idxb = wpool.tile([64, 128], F32, tag="idxb")
                        nc.gpsimd.partition_broadcast(idxb[:, 0:n], idxf[0:1, cs:cs + n])
                        ang = wpool.tile([64, 128], F32, tag="ang")
                        nc.vector.tensor_scalar_mul(ang[:, 0:n], idxb[:, 0:n], coef_t[:, 0:1])
                        q = wpool.tile([64, 128], F32, tag="q")
                        nc.vector.tensor_scalar(q[:, 0:n], ang[:, 0:n],
                                                1.0 / TWO_PI, qoff[:, 0:1], MULT, ADD)
                        qi = wpool.tile([64, 128], I32, tag="qi")
                        nc.vector.tensor_copy(qi[:, 0:n], q[:, 0:n])
                        qf = wpool.tile([64, 128], F32, tag="qf")
                        nc.vector.tensor_copy(qf[:, 0:n], qi[:, 0:n])
                        tt = wpool.tile([64, 128], F32, tag="tt")
                        nc.vector.tensor_scalar_mul(tt[:, 0:n], qf[:, 0:n], C_HI)
                        rr = wpool.tile([64, 128], F32, tag="rr")
                        nc.vector.tensor_tensor(rr[:, 0:n], ang[:, 0:n], tt[:, 0:n], SUB)
                        nc.vector.tensor_scalar_mul(tt[:, 0:n], qf[:, 0:n], C_LO)
                        nc.vector.tensor_tensor(rr[:, 0:n], rr[:, 0:n], tt[:, 0:n], SUB)
                        onedT = wpool.tile([64, 128], F32, tag="onedT")
                        nc.scalar.activation(onedT[:, 0:n], rr[:, 0:n], Sin,
                                             bias=sbias[:, 0:1])
                        nc.tensor.matmul(psS[0:n, :], onedT[:, 0:n], posW_k[k][:],
                                         start=False, stop=True)
                    sres = wpool.tile([128, 384], F32, tag="sres")
                    nc.vector.tensor_copy(sres[0:n, :], psS[0:n, :])
                    nc.scalar.dma_start(s_t[base + cs: base + cs + n, :], sres[0:n, :])

    nc.compile()
    return nc


def _get_prog():
    global _PROG
    if _PROG is None:
        _PROG = _build_program()
    return _PROG


_OH1, _OH2 = None, None


def kernel(**inputs):
    global _OH1, _OH2
    import os
    from concourse.bass_utils import run_bass_kernel_spmd

    nc = _get_prog()
    if _OH1 is None:
        _OH1, _OH2 = _onehots()
    coef = _coef_table()

    f32 = np.float32
    common = {
        "hd": np.ascontiguousarray(inputs["hd"], f32),
        "mask": np.ascontiguousarray(inputs["mask"], f32),
        "seq_W": np.ascontiguousarray(inputs["seq_W"], f32),
        "seq_b": np.ascontiguousarray(inputs["seq_b"], f32),
        "pos_W": np.ascontiguousarray(inputs["pos_W"], f32),
        "pos_b": np.ascontiguousarray(inputs["pos_b"], f32),
        "tab1": np.ascontiguousarray(inputs["tab1"], f32),
        "b1": np.ascontiguousarray(inputs["b1"], f32),
        "tab2": np.ascontiguousarray(inputs["tab2"], f32),
        "b2": np.ascontiguousarray(inputs["b2"], f32),
        "collapse_token": np.ascontiguousarray(inputs["collapse_token"], f32),
        "collapse_weight": np.ascontiguousarray(inputs["collapse_weight"], f32),
        "region_w": np.ascontiguousarray(inputs["region_w"], f32),
        "coef64": coef,
    }
    for nm in COND_NAMES:
        common[nm] = np.ascontiguousarray(inputs[nm], f32)
        common[nm + "_idx32"] = np.ascontiguousarray(inputs[nm + "_idx"]).astype(np.int32)

    in_maps = [dict(common, oh1=_OH1[c], oh2=_OH2[c]) for c in range(N_CORES)]

    trace = bool(int(os.environ.get("BASS_KERNEL_TRACE", "0")))
    last_exc = None
    for _attempt in range(3):
        try:
            res = run_bass_kernel_spmd(nc, in_maps, core_ids=list(range(N_CORES)),
                                       trace=trace)
            break
        except Exception as e:   # transient LoadExecutable failures seen on axon
            last_exc = e
    else:
        raise last_exc

    kernel.last_results = res
    z = np.concatenate([res.results[c]["z_shard"] for c in range(N_CORES)], axis=0)
    s_out = res.results[0]["s_out"]
    return s_out, z


# revision 7
# speedup vs baseline: 1.1674x; 1.0434x over previous
"""Trainium2 Bass kernel for nn_CollapseAwareEmbedding.

Output:
  s_out [1096, 384]  - tiny embedding table (computed on every core, core 0's copy used)
  z     [1096, 1096, 128] - pair grid, sharded row-wise: core c writes rows [137c, 137c+137)

z structure: z[i,j,:] = concat(tab1[pid(i,j)//4]+b1, tab2[pid(i,j)%4]+b2) where pid is a
static function of (i,j). pid is piecewise constant on an 8x8 region grid (plus a 3-wide
diagonal band inside the hd x hd block), so each core's shard is written with ~16 large
broadcast DMAs sourced from a per-(row, col-segment) vector table V built on-device by
one-hot matmuls against the runtime tables. The one-hot selectors are per-core inputs,
keeping the SPMD program identical across cores; only the diagonal-band fixups are
conditional DMAs keyed on partition_id.
"""

import math
import numpy as np

N_CORES = 8
L = 1096
SHARD = 137          # rows per core
ZC = 128             # z channel dim
ROW_ELEMS = L * ZC   # elements per z row = 140288

# region layout: collapse(1), hd(400), mhc(400), pep(15), lv(120), lj(20), hv(120), hj(20)
BOUNDS = [0, 1, 401, 801, 816, 936, 956, 1076, 1096]
SEGS = list(zip(BOUNDS[:-1], BOUNDS[1:]))   # 8 column segments
N_COND = 6
COND_NAMES = ["mhc", "pep", "lv", "lj", "hv", "hj"]
COND_LENS = [400, 15, 120, 20, 120, 20]
HD_LEN = 400
NSEG = 11            # 8 real segments + 3 patch pseudo-segments (vals 2,0,2)
PATCH_VALS = (2, 0, 2)

D_POS = 64
MAX_LEN = 2056
TWO_PI = 2.0 * math.pi
C_HI = float(np.float32(6.28125))                    # exact in f32
C_LO = float(np.float32(TWO_PI - 6.28125))


def _region_id(i):
    """0 = collapse, 1 = hd, 2+k = conditioning region k."""
    for r in range(8):
        if BOUNDS[r] <= i < BOUNDS[r + 1]:
            return r
    raise ValueError(i)


def _inter(a, b):
    # conditioning inter-region pair id, a < b, both in [0, 6)
    return 5 + N_COND + a * (N_COND - 1) - a * (a - 1) // 2 + (b - a - 1)


def _vsel_table():
    """vsel[i, s] = pid value of (row i, col segment s) away from the hd diagonal."""
    vs = np.zeros((L, 8), np.int64)
    rid = np.array([_region_id(i) for i in range(L)])
    for i in range(L):
        ri = rid[i]
        # s = 0: collapse column
        vs[i, 0] = 0 if ri == 0 else 1
        # s = 1: hd columns
        vs[i, 1] = 1 if ri == 0 else (3 if ri == 1 else 4)
        # s >= 2: conditioning region kj = s - 2
        for s in range(2, 8):
            kj = s - 2
            if ri == 0:
                vs[i, s] = 1
            elif ri == 1:
                vs[i, s] = 4
            else:
                ki = ri - 2
                if ki == kj:
                    vs[i, s] = 5 + ki
                else:
                    vs[i, s] = _inter(min(ki, kj), max(ki, kj))
    return vs


def _onehots():
    """Per-core one-hot selector tables.

    oh1[c] : [NSEG, 32, 128]  column m -> V1 partition m -> shard row m (m in [0,128))
    oh2[c] : [32, 99]         column q = 9*s + rr -> V2 row 128+rr, segment s
    """
    vs = _vsel_table()
    oh1 = np.zeros((N_CORES, NSEG, 32, 128), np.float32)
    oh2 = np.zeros((N_CORES, 32, 99), np.float32)
    for c in range(N_CORES):
        rows = np.arange(SHARD * c, SHARD * (c + 1))
        for s in range(8):
            oh1[c, s, vs[rows[:128], s], np.arange(128)] = 1.0
            for rr in range(9):
                oh2[c, vs[rows[128 + rr], s], 9 * s + rr] = 1.0
        for t, pv in enumerate(PATCH_VALS):
            oh1[c, 8 + t, pv, :] = 1.0
            oh2[c, pv, 9 * (8 + t): 9 * (8 + t) + 9] = 1.0
    return oh1, oh2


def _coef_table():
    """[64] f32: rows 0:32 sin coefs, rows 32:64 identical (cos uses +pi/2 bias)."""
    K = np.arange(32, dtype=np.float32)
    c = np.float32(math.pi) / np.power(np.float32(MAX_LEN), (2.0 * K / 64.0).astype(np.float32))
    return np.concatenate([c, c]).astype(np.float32)


def _band_specs():
    """Per-core diagonal-band fixups for the hd x hd block.

    Returns dict c -> list of ("v1", ka, kb) / ("v2", ka, kb) full 3-wide bands over
    shard rows [ka, kb), plus ("lo", k) / ("hi", k) 2-wide edges (abs rows 1 and 400).
    """
    specs = {c: [] for c in range(N_CORES)}
    for c in range(N_CORES):
        full = [k for k in range(SHARD) if 2 <= SHARD * c + k <= HD_LEN - 1]
        v1 = [k for k in full if k < 128]
        v2 = [k for k in full if k >= 128]
        if v1:
            specs[c].append(("v1", v1[0], v1[-1] + 1))
        if v2:
            specs[c].append(("v2", v2[0], v2[-1] + 1))
        for k in range(SHARD):
            if SHARD * c + k == 1:
                specs[c].append(("lo", k, k + 1))
            if SHARD * c + k == HD_LEN:
                specs[c].append(("hi", k, k + 1))
    return specs


_PROG = None


def _build_program():
    import concourse.bacc as bacc
    import concourse.tile as tile
    import concourse.bass as bass
    import concourse.mybir as mybir
    from concourse.masks import make_identity

    F32 = mybir.dt.float32
    I32 = mybir.dt.int32
    Sin = mybir.ActivationFunctionType.Sin
    SUB = mybir.AluOpType.subtract
    ADD = mybir.AluOpType.add
    MULT = mybir.AluOpType.mult

    nc = bacc.Bacc("TRN2", target_bir_lowering=False, debug=False, num_devices=N_CORES)

    # ---- inputs ----
    inp = {}
    def din(name, shape, dt=F32):
        inp[name] = nc.dram_tensor(name, shape, dt, kind="ExternalInput")
        return inp[name]

    hd_in = din("hd", [HD_LEN, 21])
    mask_in = din("mask", [HD_LEN])
    cond_ins = {}
    idx_ins = {}
    for nm, ln in zip(COND_NAMES, COND_LENS):
        cond_ins[nm] = din(nm, [ln, 21])
        idx_ins[nm] = din(nm + "_idx32", [ln], I32)
    seqW_in = din("seq_W", [22, 384])
    seqb_in = din("seq_b", [384])
    posW_in = din("pos_W", [64, 384])
    posb_in = din("pos_b", [384])
    tab1_in = din("tab1", [8, 64])
    b1_in = din("b1", [64])
    tab2_in = din("tab2", [4, 64])
    b2_in = din("b2", [64])
    ct_in = din("collapse_token", [1, 384])
    cw_in = din("collapse_weight", [1])
    rw_in = din("region_w", [6, 2])
    oh1_in = din("oh1", [NSEG, 32, 128])
    oh2_in = din("oh2", [32, 99])
    coef_in = din("coef64", [64])

    z_segs = [nc.dram_tensor(f"z_seg{si}", [SHARD, j1 - j0, ZC], F32,
                             kind="ExternalOutput")
              for si, (j0, j1) in enumerate(SEGS)]
    s_t = nc.dram_tensor("s_out", [L, 384], F32, kind="ExternalOutput")

    band = _band_specs()

    REP = 8                      # vector copies per segment in the wide tables
    SEG_ORDER = [1, 2, 4, 6, 3, 5, 7, 0]          # big segments first
    SEG_REP = {0: 1, 1: 8, 2: 8, 3: 5, 4: 8, 5: 4, 6: 8, 7: 4}   # rep | width
    # ring assignment balanced by bytes: chunk1 on opposite rings for hd/mhc etc.
    RING1 = {1: 0, 2: 1, 4: 0, 6: 1, 3: 1, 5: 0, 7: 1, 0: 0}      # chunk1: 0=sync 1=scalar
    RING2 = {s: 1 - r for s, r in RING1.items()}                   # chunk2: opposite ring

    with tile.TileContext(nc) as tc:
        with tc.tile_pool(name="consts", bufs=1) as cpool, \
             tc.tile_pool(name="work", bufs=3) as wpool, \
             tc.tile_pool(name="ps_v", bufs=3, space="PSUM") as ps_v, \
             tc.tile_pool(name="ps_t", bufs=2, space="PSUM") as ps_t:

            # ---------- LUT [32, 128]: lut[v] = [tab1[v//4]+b1 | tab2[v%4]+b2] ----------
            t1r = cpool.tile([32, 64], F32, tag="t1r")
            nc.sync.dma_start(t1r[:], tab1_in[:].unsqueeze(1).broadcast_to([8, 4, 64]))
            t2r = cpool.tile([32, 64], F32, tag="t2r")
            nc.sync.dma_start(t2r[:], tab2_in[:].unsqueeze(0).broadcast_to([8, 4, 64]))
            b1r = cpool.tile([32, 64], F32, tag="b1r")
            nc.sync.dma_start(b1r[:], b1_in[:].unsqueeze(0).broadcast_to([32, 64]))
            b2r = cpool.tile([32, 64], F32, tag="b2r")
            nc.sync.dma_start(b2r[:], b2_in[:].unsqueeze(0).broadcast_to([32, 64]))
            lut = cpool.tile([32, 128], F32, tag="lut")
            nc.vector.tensor_tensor(lut[:, 0:64], t1r[:], b1r[:], ADD)
            nc.vector.tensor_tensor(lut[:, 64:128], t2r[:], b2r[:], ADD)

            # ---------- one-hot tables ----------
            oh1_t = cpool.tile([32, NSEG * 128], F32, tag="oh1t")
            nc.sync.dma_start(oh1_t[:], oh1_in[:].transpose([1, 0, 2]))
            oh2_t = cpool.tile([32, 99], F32, tag="oh2t")
            nc.sync.dma_start(oh2_t[:], oh2_in[:])

            # lut replicated REP x along free dim -> descriptors of REP*512 B
            lutw = cpool.tile([32, REP * 128], F32, tag="lutw")
            nc.vector.tensor_copy(
                lutw[:], lut[:].unsqueeze(1).broadcast_to([32, REP, 128]))

            # ---------- V tables (per segment, wide) + bulk z DMAs ----------
            engs = [nc.sync, nc.scalar]
            Vp1 = cpool.tile([128, 3 * 128], F32, tag="Vp1")
            Vp2 = cpool.tile([9, 3 * 128], F32, tag="Vp2")

            for si in SEG_ORDER:
                j0, j1 = SEGS[si]
                w = j1 - j0
                rep = SEG_REP[si]
                vw = cpool.tile([128, REP * 128], F32, tag=f"vw{si}")
                psv = ps_v.tile([128, REP * 128], F32, tag="v")
                for h in range(REP * 128 // 512):
                    nc.tensor.matmul(psv[:, h * 512:(h + 1) * 512],
                                     oh1_t[:, si * 128:(si + 1) * 128],
                                     lutw[:, h * 512:(h + 1) * 512],
                                     start=True, stop=True)
                nc.vector.tensor_copy(vw[:], psv[:])
                src1 = vw[:, 0:rep * 128].unsqueeze(1).broadcast_to(
                    [128, w // rep, rep * 128])
                engs[RING1[si]].dma_start(z_segs[si][0:128, :, :], src1)

                v2 = cpool.tile([9, REP * 128], F32, tag=f"v2_{si}")
                psv2 = ps_v.tile([9, REP * 128], F32, tag="v")
                for h in range(REP * 128 // 512):
                    nc.tensor.matmul(psv2[:, h * 512:(h + 1) * 512],
                                     oh2_t[:, si * 9: si * 9 + 9],
                                     lutw[:, h * 512:(h + 1) * 512],
                                     start=True, stop=True)
                nc.vector.tensor_copy(v2[:], psv2[:])
                src2 = v2[:, 0:rep * 128].unsqueeze(1).broadcast_to(
                    [9, w // rep, rep * 128])
                engs[RING2[si]].dma_start(z_segs[si][128:SHARD, :, :], src2)

            # contiguous 3-vector diagonal patches (vals 2,0,2)
            psP = ps_v.tile([128, 3 * 128], F32, tag="v")
            for t in range(3):
                nc.tensor.matmul(psP[0:128, t * 128:(t + 1) * 128],
                                 oh1_t[:, (8 + t) * 128:(9 + t) * 128], lut[:],
                                 start=True, stop=True)
            nc.vector.tensor_copy(Vp1[:], psP[:])
            psP2 = ps_v.tile([9, 3 * 128], F32, tag="v")
            for t in range(3):
                nc.tensor.matmul(psP2[0:9, t * 128:(t + 1) * 128],
                                 oh2_t[:, 9 * (8 + t): 9 * (8 + t) + 9], lut[:],
                                 start=True, stop=True)
            nc.vector.tensor_copy(Vp2[:], psP2[:])

            # ---------- conditional diagonal-band fixups (z_seg1, local col = j-1) ----------
            ROW1 = (SEGS[1][1] - SEGS[1][0]) * ZC   # 400*128 elements per seg1 row
            zs1 = z_segs[1]
            pid_sp = nc.sync.partition_id()
            eqs = {c: (pid_sp == c) for c in range(N_CORES) if band[c]}
            for c, items in band.items():
                if not items:
                    continue
                eq = eqs[c]
                for kind, ka, kb in items:
                    n = kb - ka
                    if kind in ("v1", "v2"):
                        off = ka * ROW1 + (SHARD * c + ka - 2) * ZC
                        dst = bass.AP(tensor=zs1, offset=off,
                                      ap=[[ROW1 + ZC, n], [1, 3 * ZC]])
                        src_ap = Vp1[ka:kb, 0:384] if kind == "v1" else Vp2[ka - 128:kb - 128, :]
                        dst = nc.ap_or_oob(dst, eq)
                        nc.sync.dma_start(dst, src_ap, bounds_check="skip_entire_dma")
                    elif kind == "lo":   # abs row 1: local cols (0,1) = vals (0,2)
                        dst = nc.ap_or_oob(zs1[ka:kb, 0:2, :], eq)
                        nc.sync.dma_start(dst, Vp1[ka:kb, 128:384],
                                          bounds_check="skip_entire_dma")
                    else:                # abs row 400: local cols (398,399) = vals (2,0)
                        dst = nc.ap_or_oob(zs1[ka:kb, 398:400, :], eq)
                        nc.sync.dma_start(dst, Vp1[ka:kb, 0:256],
                                          bounds_check="skip_entire_dma")

            # ---------- s_out ----------
            ident = cpool.tile([128, 128], F32, tag="ident")
            make_identity(nc, ident[:])

            seqWb = cpool.tile([23, 384], F32, tag="seqWb")   # row 0 = seq_b, rows 1:23 = seq_W
            nc.gpsimd.dma_start(seqWb[0:1, :], seqb_in[:].unsqueeze(0))
            nc.gpsimd.dma_start(seqWb[1:23, :], seqW_in[:])
            posW = cpool.tile([64, 384], F32, tag="posW")
            nc.gpsimd.dma_start(posW[:], posW_in[:])
            posb = cpool.tile([1, 384], F32, tag="posb")
            nc.gpsimd.dma_start(posb[:], posb_in[:].unsqueeze(0))
            coef_t = cpool.tile([64, 1], F32, tag="coef")
            nc.gpsimd.dma_start(coef_t[:], coef_in[:].rearrange("(p one) -> p one", one=1))
            qoff = cpool.tile([64, 1], F32, tag="qoff")
            nc.vector.memset(qoff[0:32], 0.0)
            nc.vector.memset(qoff[32:64], 0.25)
            sbias = cpool.tile([64, 1], F32, tag="sbias")
            nc.vector.memset(sbias[0:32], 0.0)
            nc.vector.memset(sbias[32:64], math.pi / 2.0)

            # region weights, scaled rhs tables
            rhs_k = {}
            posW_k = {}
            for k in range(N_COND):
                w0 = cpool.tile([128, 1], F32, tag=f"w0_{k}")
                nc.gpsimd.dma_start(
                    w0[:], rw_in[k:k + 1, 0:1].broadcast_to([128, 1]))
                w1 = cpool.tile([128, 1], F32, tag=f"w1_{k}")
                nc.gpsimd.dma_start(
                    w1[:], rw_in[k:k + 1, 1:2].broadcast_to([128, 1]))
                rk = cpool.tile([23, 384], F32, tag=f"rhs_{k}")
                nc.vector.tensor_scalar_mul(rk[:], seqWb[:], w0[0:23, 0:1])
                tmp = wpool.tile([1, 384], F32, tag="rtmp")
                nc.vector.tensor_scalar_mul(tmp[:], posb[:], w1[0:1, 0:1])
                nc.vector.tensor_tensor(rk[0:1, :], rk[0:1, :], tmp[:], ADD)
                rhs_k[k] = rk
                pk = cpool.tile([64, 384], F32, tag=f"posWk_{k}")
                nc.vector.tensor_scalar_mul(pk[:], posW[:], w1[0:64, 0:1])
                posW_k[k] = pk

            # collapse row
            ct = cpool.tile([1, 384], F32, tag="ct")
            nc.gpsimd.dma_start(ct[:], ct_in[:])
            cw = cpool.tile([1, 1], F32, tag="cw")
            nc.gpsimd.dma_start(cw[:], cw_in[:].unsqueeze(0))
            s0 = cpool.tile([1, 384], F32, tag="s0")
            nc.vector.tensor_scalar_mul(s0[:], ct[:], cw[0:1, 0:1])
            nc.gpsimd.dma_start(s_t[0:1, :], s0[:])

            # per-region chunks
            regions = [("hd", HD_LEN, 1, None)] + [
                (nm, ln, BOUNDS[2 + k], k) for k, (nm, ln) in enumerate(zip(COND_NAMES, COND_LENS))
            ]
            for nm, ln, base, k in regions:
                is_hd = k is None
                x_in = hd_in if is_hd else cond_ins[nm]
                if not is_hd:
                    idxi = cpool.tile([1, ln], I32, tag=f"idxi_{nm}")
                    nc.gpsimd.dma_start(idxi[:], idx_ins[nm][:].unsqueeze(0))
                    idxf = cpool.tile([1, ln], F32, tag=f"idxf_{nm}")
                    nc.vector.tensor_copy(idxf[:], idxi[:])
                for cs in range(0, ln, 128):
                    n = min(128, ln - cs)
                    aa = wpool.tile([128, 23], F32, tag="aa")
                    nc.vector.memset(aa[0:n, 0:1], 1.0)
                    if is_hd:
                        nc.gpsimd.dma_start(
                            aa[0:n, 1:2],
                            mask_in[cs:cs + n].rearrange("(p one) -> p one", one=1))
                    else:
                        nc.vector.memset(aa[0:n, 1:2], 0.0)
                    nc.gpsimd.dma_start(aa[0:n, 2:23], x_in[cs:cs + n, :])
                    psT = ps_t.tile([23, 128], F32)
                    nc.tensor.transpose(psT[0:23, 0:n], aa[0:n, 0:23], ident[0:n, 0:n])
                    aaT = wpool.tile([23, 128], F32, tag="aaT")
                    nc.vector.tensor_copy(aaT[0:23, 0:n], psT[0:23, 0:n])
                    psS = ps_v.tile([128, 384], F32, tag="v")
                    nc.tensor.matmul(psS[0:n, :], aaT[0:23, 0:n],
                                     seqWb[:] if is_hd else rhs_k[k][:],
                                     start=True, stop=is_hd)
                    if not is_hd:
                        idxb = wpool.tile([64, 128], F32, tag="idxb")
                        nc.gpsimd.partition_broadcast(idxb[:, 0:n], idxf[0:1, cs:cs + n])
                        ang = wpool.tile([64, 128], F32, tag="ang")
                        nc.vector.tensor_scalar_mul(ang[:, 0:n], idxb[:, 0:n], coef_t[:, 0:1])
                        q = wpool.tile([64, 128], F32, tag="q")
                        nc.vector.tensor_scalar(q[:, 0:n], ang[:, 0:n],
                                                1.0 / TWO_PI, qoff[:, 0:1], MULT, ADD)
                        qi = wpool.tile([64, 128], I32, tag="qi")
                        nc.vector.tensor_copy(qi[:, 0:n], q[:, 0:n])
                        qf = wpool.tile([64, 128], F32, tag="qf")
                        nc.vector.tensor_copy(qf[:, 0:n], qi[:, 0:n])
                        tt = wpool.tile([64, 128], F32, tag="tt")
                        nc.vector.tensor_scalar_mul(tt[:, 0:n], qf[:, 0:n], C_HI)
                        rr = wpool.tile([64, 128], F32, tag="rr")
                        nc.vector.tensor_tensor(rr[:, 0:n], ang[:, 0:n], tt[:, 0:n], SUB)
                        nc.vector.tensor_scalar_mul(tt[:, 0:n], qf[:, 0:n], C_LO)
                        nc.vector.tensor_tensor(rr[:, 0:n], rr[:, 0:n], tt[:, 0:n], SUB)
                        onedT = wpool.tile([64, 128], F32, tag="onedT")
                        nc.scalar.activation(onedT[:, 0:n], rr[:, 0:n], Sin,
                                             bias=sbias[:, 0:1])
                        nc.tensor.matmul(psS[0:n, :], onedT[:, 0:n], posW_k[k][:],
                                         start=False, stop=True)
                    sres = wpool.tile([128, 384], F32, tag="sres")
                    nc.vector.tensor_copy(sres[0:n, :], psS[0:n, :])
                    nc.gpsimd.dma_start(s_t[base + cs: base + cs + n, :], sres[0:n, :])

    nc.compile()
    return nc


def _get_prog():
    global _PROG
    if _PROG is None:
        _PROG = _build_program()
    return _PROG


_OH1, _OH2 = None, None


def kernel(**inputs):
    global _OH1, _OH2
    import os
    from concourse.bass_utils import run_bass_kernel_spmd

    nc = _get_prog()
    if _OH1 is None:
        _OH1, _OH2 = _onehots()
    coef = _coef_table()

    f32 = np.float32
    common = {
        "hd": np.ascontiguousarray(inputs["hd"], f32),
        "mask": np.ascontiguousarray(inputs["mask"], f32),
        "seq_W": np.ascontiguousarray(inputs["seq_W"], f32),
        "seq_b": np.ascontiguousarray(inputs["seq_b"], f32),
        "pos_W": np.ascontiguousarray(inputs["pos_W"], f32),
        "pos_b": np.ascontiguousarray(inputs["pos_b"], f32),
        "tab1": np.ascontiguousarray(inputs["tab1"], f32),
        "b1": np.ascontiguousarray(inputs["b1"], f32),
        "tab2": np.ascontiguousarray(inputs["tab2"], f32),
        "b2": np.ascontiguousarray(inputs["b2"], f32),
        "collapse_token": np.ascontiguousarray(inputs["collapse_token"], f32),
        "collapse_weight": np.ascontiguousarray(inputs["collapse_weight"], f32),
        "region_w": np.ascontiguousarray(inputs["region_w"], f32),
        "coef64": coef,
    }
    for nm in COND_NAMES:
        common[nm] = np.ascontiguousarray(inputs[nm], f32)
        common[nm + "_idx32"] = np.ascontiguousarray(inputs[nm + "_idx"]).astype(np.int32)

    in_maps = [dict(common, oh1=_OH1[c], oh2=_OH2[c]) for c in range(N_CORES)]

    trace = bool(int(os.environ.get("BASS_KERNEL_TRACE", "0")))
    last_exc = None
    for _attempt in range(3):
        try:
            res = run_bass_kernel_spmd(nc, in_maps, core_ids=list(range(N_CORES)),
                                       trace=trace)
            break
        except Exception as e:   # transient LoadExecutable failures seen on axon
            last_exc = e
    else:
        raise last_exc

    kernel.last_results = res
    z = np.empty((L, L, ZC), np.float32)
    for c in range(N_CORES):
        r0 = SHARD * c
        for si, (j0, j1) in enumerate(SEGS):
            z[r0:r0 + SHARD, j0:j1] = res.results[c][f"z_seg{si}"]
    s_out = res.results[0]["s_out"]
    return s_out, z


# revision 8
# speedup vs baseline: 1.6345x; 1.4000x over previous
"""Trainium2 Bass kernel for nn_CollapseAwareEmbedding.

Output:
  s_out [1096, 384]  - tiny embedding table (computed on every core, core 0's copy used)
  z     [1096, 1096, 128] - pair grid, sharded row-wise: core c writes rows [137c, 137c+137)

z structure: z[i,j,:] = concat(tab1[pid(i,j)//4]+b1, tab2[pid(i,j)%4]+b2) where pid is a
static function of (i,j). pid is piecewise constant on an 8x8 region grid (plus a 3-wide
diagonal band inside the hd x hd block), so each core's shard is written with ~16 large
broadcast DMAs sourced from a per-(row, col-segment) vector table V built on-device by
one-hot matmuls against the runtime tables. The one-hot selectors are per-core inputs,
keeping the SPMD program identical across cores; only the diagonal-band fixups are
conditional DMAs keyed on partition_id.
"""

import math
import numpy as np

N_CORES = 8
L = 1096
SHARD = 137          # rows per core
ZC = 128             # z channel dim
ROW_ELEMS = L * ZC   # elements per z row = 140288

# region layout: collapse(1), hd(400), mhc(400), pep(15), lv(120), lj(20), hv(120), hj(20)
BOUNDS = [0, 1, 401, 801, 816, 936, 956, 1076, 1096]
SEGS = list(zip(BOUNDS[:-1], BOUNDS[1:]))   # 8 column segments
N_COND = 6
COND_NAMES = ["mhc", "pep", "lv", "lj", "hv", "hj"]
COND_LENS = [400, 15, 120, 20, 120, 20]
HD_LEN = 400
NSEG = 11            # 8 real segments + 3 patch pseudo-segments (vals 2,0,2)
PATCH_VALS = (2, 0, 2)
# chunk2 (shard rows 128..136): seg si split into G_si equal column slices, each
# owned by one partition; partition p = rr*G + g holds row 128+rr's vector
SEG_G = {0: 1, 1: 10, 2: 10, 3: 5, 4: 10, 5: 10, 6: 10, 7: 10}
OHG_OFF = {}
_off = 0
for _si in range(8):
    OHG_OFF[_si] = _off
    _off += 9 * SEG_G[_si]
OHG_COLS = _off

D_POS = 64
MAX_LEN = 2056
TWO_PI = 2.0 * math.pi
C_HI = float(np.float32(6.28125))                    # exact in f32
C_LO = float(np.float32(TWO_PI - 6.28125))


def _region_id(i):
    """0 = collapse, 1 = hd, 2+k = conditioning region k."""
    for r in range(8):
        if BOUNDS[r] <= i < BOUNDS[r + 1]:
            return r
    raise ValueError(i)


def _inter(a, b):
    # conditioning inter-region pair id, a < b, both in [0, 6)
    return 5 + N_COND + a * (N_COND - 1) - a * (a - 1) // 2 + (b - a - 1)


def _vsel_table():
    """vsel[i, s] = pid value of (row i, col segment s) away from the hd diagonal."""
    vs = np.zeros((L, 8), np.int64)
    rid = np.array([_region_id(i) for i in range(L)])
    for i in range(L):
        ri = rid[i]
        # s = 0: collapse column
        vs[i, 0] = 0 if ri == 0 else 1
        # s = 1: hd columns
        vs[i, 1] = 1 if ri == 0 else (3 if ri == 1 else 4)
        # s >= 2: conditioning region kj = s - 2
        for s in range(2, 8):
            kj = s - 2
            if ri == 0:
                vs[i, s] = 1
            elif ri == 1:
                vs[i, s] = 4
            else:
                ki = ri - 2
                if ki == kj:
                    vs[i, s] = 5 + ki
                else:
                    vs[i, s] = _inter(min(ki, kj), max(ki, kj))
    return vs


def _onehots():
    """Per-core one-hot selector tables.

    oh1[c] : [NSEG, 32, 128]  column m -> V1 partition m -> shard row m (m in [0,128))
    oh2[c] : [32, 99]         column q = 9*s + rr -> V2 row 128+rr, segment s
    """
    vs = _vsel_table()
    oh1 = np.zeros((N_CORES, NSEG, 32, 128), np.float32)
    oh2 = np.zeros((N_CORES, 32, 27), np.float32)     # band patch rows only
    ohg = np.zeros((N_CORES, 32, OHG_COLS), np.float32)
    for c in range(N_CORES):
        rows = np.arange(SHARD * c, SHARD * (c + 1))
        for s in range(8):
            oh1[c, s, vs[rows[:128], s], np.arange(128)] = 1.0
            G = SEG_G[s]
            for rr in range(9):
                v = vs[rows[128 + rr], s]
                ohg[c, v, OHG_OFF[s] + rr * G: OHG_OFF[s] + (rr + 1) * G] = 1.0
        for t, pv in enumerate(PATCH_VALS):
            oh1[c, 8 + t, pv, :] = 1.0
            oh2[c, pv, 9 * t: 9 * t + 9] = 1.0
    return oh1, oh2, ohg


def _coef_table():
    """[64] f32: rows 0:32 sin coefs, rows 32:64 identical (cos uses +pi/2 bias)."""
    K = np.arange(32, dtype=np.float32)
    c = np.float32(math.pi) / np.power(np.float32(MAX_LEN), (2.0 * K / 64.0).astype(np.float32))
    return np.concatenate([c, c]).astype(np.float32)


def _band_specs():
    """Per-core diagonal-band fixups for the hd x hd block.

    Returns dict c -> list of ("v1", ka, kb) / ("v2", ka, kb) full 3-wide bands over
    shard rows [ka, kb), plus ("lo", k) / ("hi", k) 2-wide edges (abs rows 1 and 400).
    """
    specs = {c: [] for c in range(N_CORES)}
    for c in range(N_CORES):
        full = [k for k in range(SHARD) if 2 <= SHARD * c + k <= HD_LEN - 1]
        v1 = [k for k in full if k < 128]
        v2 = [k for k in full if k >= 128]
        if v1:
            specs[c].append(("v1", v1[0], v1[-1] + 1))
        if v2:
            specs[c].append(("v2", v2[0], v2[-1] + 1))
        for k in range(SHARD):
            if SHARD * c + k == 1:
                specs[c].append(("lo", k, k + 1))
            if SHARD * c + k == HD_LEN:
                specs[c].append(("hi", k, k + 1))
    return specs


_PROG = None


def _build_program():
    import concourse.bacc as bacc
    import concourse.tile as tile
    import concourse.bass as bass
    import concourse.mybir as mybir
    from concourse.masks import make_identity

    F32 = mybir.dt.float32
    I32 = mybir.dt.int32
    Sin = mybir.ActivationFunctionType.Sin
    SUB = mybir.AluOpType.subtract
    ADD = mybir.AluOpType.add
    MULT = mybir.AluOpType.mult

    nc = bacc.Bacc("TRN2", target_bir_lowering=False, debug=False, num_devices=N_CORES)

    # ---- inputs ----
    inp = {}
    def din(name, shape, dt=F32):
        inp[name] = nc.dram_tensor(name, shape, dt, kind="ExternalInput")
        return inp[name]

    hd_in = din("hd", [HD_LEN, 21])
    mask_in = din("mask", [HD_LEN])
    cond_ins = {}
    idx_ins = {}
    for nm, ln in zip(COND_NAMES, COND_LENS):
        cond_ins[nm] = din(nm, [ln, 21])
        idx_ins[nm] = din(nm + "_idx32", [ln], I32)
    seqW_in = din("seq_W", [22, 384])
    seqb_in = din("seq_b", [384])
    posW_in = din("pos_W", [64, 384])
    posb_in = din("pos_b", [384])
    tab1_in = din("tab1", [8, 64])
    b1_in = din("b1", [64])
    tab2_in = din("tab2", [4, 64])
    b2_in = din("b2", [64])
    ct_in = din("collapse_token", [1, 384])
    cw_in = din("collapse_weight", [1])
    rw_in = din("region_w", [6, 2])
    oh1_in = din("oh1", [NSEG, 32, 128])
    oh2_in = din("oh2", [32, 27])
    ohg_in = din("ohg", [32, OHG_COLS])
    coef_in = din("coef64", [64])

    z_segs = [nc.dram_tensor(f"z_seg{si}", [SHARD, j1 - j0, ZC], F32,
                             kind="ExternalOutput")
              for si, (j0, j1) in enumerate(SEGS)]
    s_t = nc.dram_tensor("s_out", [L, 384], F32, kind="ExternalOutput")

    band = _band_specs()

    REP = 8                      # vector copies per segment in the wide tables
    SEG_ORDER = [1, 2, 4, 6, 3, 5, 7, 0]          # big segments first
    SEG_REP = {0: 1, 1: 8, 2: 8, 3: 5, 4: 8, 5: 4, 6: 8, 7: 4}   # rep | width
    # ring assignment balanced by bytes: chunk1 on opposite rings for hd/mhc etc.
    RING1 = {1: 0, 2: 1, 4: 0, 6: 1, 3: 1, 5: 0, 7: 1, 0: 0}      # chunk1: 0=sync 1=scalar
    RING2 = {s: 1 - r for s, r in RING1.items()}                   # chunk2: opposite ring

    with tile.TileContext(nc) as tc:
        with tc.tile_pool(name="consts", bufs=1) as cpool, \
             tc.tile_pool(name="work", bufs=3) as wpool, \
             tc.tile_pool(name="ps_v", bufs=3, space="PSUM") as ps_v, \
             tc.tile_pool(name="ps_t", bufs=2, space="PSUM") as ps_t:

            # ---------- LUT [32, 128]: lut[v] = [tab1[v//4]+b1 | tab2[v%4]+b2] ----------
            t1r = cpool.tile([32, 64], F32, tag="t1r")
            nc.gpsimd.dma_start(t1r[:], tab1_in[:].unsqueeze(1).broadcast_to([8, 4, 64]))
            t2r = cpool.tile([32, 64], F32, tag="t2r")
            nc.gpsimd.dma_start(t2r[:], tab2_in[:].unsqueeze(0).broadcast_to([8, 4, 64]))
            b1r = cpool.tile([32, 64], F32, tag="b1r")
            nc.gpsimd.dma_start(b1r[:], b1_in[:].unsqueeze(0).broadcast_to([32, 64]))
            b2r = cpool.tile([32, 64], F32, tag="b2r")
            nc.gpsimd.dma_start(b2r[:], b2_in[:].unsqueeze(0).broadcast_to([32, 64]))
            lut = cpool.tile([32, 128], F32, tag="lut")
            nc.vector.tensor_tensor(lut[:, 0:64], t1r[:], b1r[:], ADD)
            nc.vector.tensor_tensor(lut[:, 64:128], t2r[:], b2r[:], ADD)

            # ---------- one-hot tables ----------
            oh1_t = cpool.tile([32, NSEG * 128], F32, tag="oh1t")
            nc.gpsimd.dma_start(oh1_t[:], oh1_in[:].transpose([1, 0, 2]))
            oh2_t = cpool.tile([32, 27], F32, tag="oh2t")
            ohg_t = cpool.tile([32, OHG_COLS], F32, tag="ohgt")
            nc.gpsimd.dma_start(ohg_t[:], ohg_in[:])
            nc.gpsimd.dma_start(oh2_t[:], oh2_in[:])

            # lut replicated REP x along free dim -> descriptors of REP*512 B
            lutw = cpool.tile([32, REP * 128], F32, tag="lutw")
            nc.vector.tensor_copy(
                lutw[:], lut[:].unsqueeze(1).broadcast_to([32, REP, 128]))

            # ---------- V tables (per segment, wide) ----------
            Vp1 = cpool.tile([128, 3 * 128], F32, tag="Vp1")
            Vp2 = cpool.tile([9, 3 * 128], F32, tag="Vp2")

            vws = {}
            v2gs = {}
            for si in SEG_ORDER:
                j0, j1 = SEGS[si]
                w = j1 - j0
                vw = cpool.tile([128, REP * 128], F32, tag=f"vw{si}")
                psv = ps_v.tile([128, REP * 128], F32, tag="v")
                for h in range(REP * 128 // 512):
                    nc.tensor.matmul(psv[:, h * 512:(h + 1) * 512],
                                     oh1_t[:, si * 128:(si + 1) * 128],
                                     lutw[:, h * 512:(h + 1) * 512],
                                     start=True, stop=True)
                nc.vector.tensor_copy(vw[:], psv[:])
                vws[si] = vw

                G = SEG_G[si]
                R2 = w // G
                v2g = cpool.tile([9 * G, R2 * 128], F32, tag=f"v2g{si}")
                pv2 = ps_v.tile([9 * G, 128], F32, tag="v")
                nc.tensor.matmul(pv2[:], ohg_t[:, OHG_OFF[si]: OHG_OFF[si] + 9 * G],
                                 lut[:], start=True, stop=True)
                nc.vector.tensor_copy(
                    v2g[:], pv2[:].unsqueeze(1).broadcast_to([9 * G, R2, 128]))
                v2gs[si] = v2g

            # contiguous 3-vector diagonal patches (vals 2,0,2)
            psP = ps_v.tile([128, 3 * 128], F32, tag="v")
            for t in range(3):
                nc.tensor.matmul(psP[0:128, t * 128:(t + 1) * 128],
                                 oh1_t[:, (8 + t) * 128:(9 + t) * 128], lut[:],
                                 start=True, stop=True)
            nc.vector.tensor_copy(Vp1[:], psP[:])
            psP2 = ps_v.tile([9, 3 * 128], F32, tag="v")
            for t in range(3):
                nc.tensor.matmul(psP2[0:9, t * 128:(t + 1) * 128],
                                 oh2_t[:, 9 * t: 9 * t + 9], lut[:],
                                 start=True, stop=True)
            nc.vector.tensor_copy(Vp2[:], psP2[:])

            # ---------- s_out ----------
            ident = cpool.tile([128, 128], F32, tag="ident")
            make_identity(nc, ident[:])

            seqWb = cpool.tile([23, 384], F32, tag="seqWb")   # row 0 = seq_b, rows 1:23 = seq_W
            nc.gpsimd.dma_start(seqWb[0:1, :], seqb_in[:].unsqueeze(0))
            nc.gpsimd.dma_start(seqWb[1:23, :], seqW_in[:])
            posW = cpool.tile([64, 384], F32, tag="posW")
            nc.gpsimd.dma_start(posW[:], posW_in[:])
            posb = cpool.tile([1, 384], F32, tag="posb")
            nc.gpsimd.dma_start(posb[:], posb_in[:].unsqueeze(0))
            coef_t = cpool.tile([64, 1], F32, tag="coef")
            nc.gpsimd.dma_start(coef_t[:], coef_in[:].rearrange("(p one) -> p one", one=1))
            qoff = cpool.tile([64, 1], F32, tag="qoff")
            nc.vector.memset(qoff[0:32], 0.0)
            nc.vector.memset(qoff[32:64], 0.25)
            sbias = cpool.tile([64, 1], F32, tag="sbias")
            nc.vector.memset(sbias[0:32], 0.0)
            nc.vector.memset(sbias[32:64], math.pi / 2.0)

            # region weights, scaled rhs tables
            rhs_k = {}
            posW_k = {}
            for k in range(N_COND):
                w0 = cpool.tile([128, 1], F32, tag=f"w0_{k}")
                nc.gpsimd.dma_start(
                    w0[:], rw_in[k:k + 1, 0:1].broadcast_to([128, 1]))
                w1 = cpool.tile([128, 1], F32, tag=f"w1_{k}")
                nc.gpsimd.dma_start(
                    w1[:], rw_in[k:k + 1, 1:2].broadcast_to([128, 1]))
                rk = cpool.tile([23, 384], F32, tag=f"rhs_{k}")
                nc.vector.tensor_scalar_mul(rk[:], seqWb[:], w0[0:23, 0:1])
                tmp = wpool.tile([1, 384], F32, tag="rtmp")
                nc.vector.tensor_scalar_mul(tmp[:], posb[:], w1[0:1, 0:1])
                nc.vector.tensor_tensor(rk[0:1, :], rk[0:1, :], tmp[:], ADD)
                rhs_k[k] = rk
                pk = cpool.tile([64, 384], F32, tag=f"posWk_{k}")
                nc.vector.tensor_scalar_mul(pk[:], posW[:], w1[0:64, 0:1])
                posW_k[k] = pk

            # collapse row
            ct = cpool.tile([1, 384], F32, tag="ct")
            nc.gpsimd.dma_start(ct[:], ct_in[:])
            cw = cpool.tile([1, 1], F32, tag="cw")
            nc.gpsimd.dma_start(cw[:], cw_in[:].unsqueeze(0))
            s0 = cpool.tile([1, 384], F32, tag="s0")
            nc.vector.tensor_scalar_mul(s0[:], ct[:], cw[0:1, 0:1])
            nc.gpsimd.dma_start(s_t[0:1, :], s0[:])

            # per-region chunks
            regions = [("hd", HD_LEN, 1, None)] + [
                (nm, ln, BOUNDS[2 + k], k) for k, (nm, ln) in enumerate(zip(COND_NAMES, COND_LENS))
            ]
            for nm, ln, base, k in regions:
                is_hd = k is None
                x_in = hd_in if is_hd else cond_ins[nm]
                if not is_hd:
                    idxi = cpool.tile([1, ln], I32, tag=f"idxi_{nm}")
                    nc.gpsimd.dma_start(idxi[:], idx_ins[nm][:].unsqueeze(0))
                    idxf = cpool.tile([1, ln], F32, tag=f"idxf_{nm}")
                    nc.vector.tensor_copy(idxf[:], idxi[:])
                for cs in range(0, ln, 128):
                    n = min(128, ln - cs)
                    aa = wpool.tile([128, 23], F32, tag="aa")
                    nc.vector.memset(aa[0:n, 0:1], 1.0)
                    if is_hd:
                        nc.gpsimd.dma_start(
                            aa[0:n, 1:2],
                            mask_in[cs:cs + n].rearrange("(p one) -> p one", one=1))
                    else:
                        nc.vector.memset(aa[0:n, 1:2], 0.0)
                    nc.gpsimd.dma_start(aa[0:n, 2:23], x_in[cs:cs + n, :])
                    psT = ps_t.tile([23, 128], F32)
                    nc.tensor.transpose(psT[0:23, 0:n], aa[0:n, 0:23], ident[0:n, 0:n])
                    aaT = wpool.tile([23, 128], F32, tag="aaT")
                    nc.vector.tensor_copy(aaT[0:23, 0:n], psT[0:23, 0:n])
                    psS = ps_v.tile([128, 384], F32, tag="v")
                    nc.tensor.matmul(psS[0:n, :], aaT[0:23, 0:n],
                                     seqWb[:] if is_hd else rhs_k[k][:],
                                     start=True, stop=is_hd)
                    if not is_hd:
                        idxb = wpool.tile([64, 128], F32, tag="idxb")
                        nc.gpsimd.partition_broadcast(idxb[:, 0:n], idxf[0:1, cs:cs + n])
                        ang = wpool.tile([64, 128], F32, tag="ang")
                        nc.vector.tensor_scalar_mul(ang[:, 0:n], idxb[:, 0:n], coef_t[:, 0:1])
                        q = wpool.tile([64, 128], F32, tag="q")
                        nc.vector.tensor_scalar(q[:, 0:n], ang[:, 0:n],
                                                1.0 / TWO_PI, qoff[:, 0:1], MULT, ADD)
                        qi = wpool.tile([64, 128], I32, tag="qi")
                        nc.vector.tensor_copy(qi[:, 0:n], q[:, 0:n])
                        qf = wpool.tile([64, 128], F32, tag="qf")
                        nc.vector.tensor_copy(qf[:, 0:n], qi[:, 0:n])
                        tt = wpool.tile([64, 128], F32, tag="tt")
                        nc.vector.tensor_scalar_mul(tt[:, 0:n], qf[:, 0:n], C_HI)
                        rr = wpool.tile([64, 128], F32, tag="rr")
                        nc.vector.tensor_tensor(rr[:, 0:n], ang[:, 0:n], tt[:, 0:n], SUB)
                        nc.vector.tensor_scalar_mul(tt[:, 0:n], qf[:, 0:n], C_LO)
                        nc.vector.tensor_tensor(rr[:, 0:n], rr[:, 0:n], tt[:, 0:n], SUB)
                        onedT = wpool.tile([64, 128], F32, tag="onedT")
                        nc.scalar.activation(onedT[:, 0:n], rr[:, 0:n], Sin,
                                             bias=sbias[:, 0:1])
                        nc.tensor.matmul(psS[0:n, :], onedT[:, 0:n], posW_k[k][:],
                                         start=False, stop=True)
                    sres = wpool.tile([128, 384], F32, tag="sres")
                    nc.vector.tensor_copy(sres[0:n, :], psS[0:n, :])
                    nc.gpsimd.dma_start(s_t[base + cs: base + cs + n, :], sres[0:n, :])


            # ---------- bulk z DMAs (after s_out emission so Scalar's Sins run first) ----------
            engs = [nc.sync, nc.scalar]
            for si in SEG_ORDER:
                j0, j1 = SEGS[si]
                w = j1 - j0
                rep = SEG_REP[si]
                src1 = vws[si][:, 0:rep * 128].unsqueeze(1).broadcast_to(
                    [128, w // rep, rep * 128])
                engs[RING1[si]].dma_start(z_segs[si][0:128, :, :], src1)
                G = SEG_G[si]
                R2 = w // G
                dst2 = z_segs[si][128:SHARD, :, :].flatten().rearrange(
                    "(p f) -> p f", f=R2 * 128)
                engs[RING2[si]].dma_start(dst2, v2gs[si][:, 0:R2 * 128])

            # ---------- conditional diagonal-band fixups (z_seg1, local col = j-1) ----------
            ROW1 = (SEGS[1][1] - SEGS[1][0]) * ZC   # 400*128 elements per seg1 row
            zs1 = z_segs[1]
            pid_sp = nc.sync.partition_id()
            eqs = {c: (pid_sp == c) for c in range(N_CORES) if band[c]}
            for c, items in band.items():
                if not items:
                    continue
                eq = eqs[c]
                for kind, ka, kb in items:
                    n = kb - ka
                    if kind in ("v1", "v2"):
                        off = ka * ROW1 + (SHARD * c + ka - 2) * ZC
                        dst = bass.AP(tensor=zs1, offset=off,
                                      ap=[[ROW1 + ZC, n], [1, 3 * ZC]])
                        src_ap = Vp1[ka:kb, 0:384] if kind == "v1" else Vp2[ka - 128:kb - 128, :]
                        dst = nc.ap_or_oob(dst, eq)
                        nc.sync.dma_start(dst, src_ap, bounds_check="skip_entire_dma")
                    elif kind == "lo":   # abs row 1: local cols (0,1) = vals (0,2)
                        dst = nc.ap_or_oob(zs1[ka:kb, 0:2, :], eq)
                        nc.sync.dma_start(dst, Vp1[ka:kb, 128:384],
                                          bounds_check="skip_entire_dma")
                    else:                # abs row 400: local cols (398,399) = vals (2,0)
                        dst = nc.ap_or_oob(zs1[ka:kb, 398:400, :], eq)
                        nc.sync.dma_start(dst, Vp1[ka:kb, 0:256],
                                          bounds_check="skip_entire_dma")

    nc.compile()
    return nc


def _get_prog():
    global _PROG
    if _PROG is None:
        _PROG = _build_program()
    return _PROG


_OH1, _OH2, _OHG = None, None, None


def kernel(**inputs):
    global _OH1, _OH2, _OHG
    import os
    from concourse.bass_utils import run_bass_kernel_spmd

    nc = _get_prog()
    if _OH1 is None:
        _OH1, _OH2, _OHG = _onehots()
    coef = _coef_table()

    f32 = np.float32
    common = {
        "hd": np.ascontiguousarray(inputs["hd"], f32),
        "mask": np.ascontiguousarray(inputs["mask"], f32),
        "seq_W": np.ascontiguousarray(inputs["seq_W"], f32),
        "seq_b": np.ascontiguousarray(inputs["seq_b"], f32),
        "pos_W": np.ascontiguousarray(inputs["pos_W"], f32),
        "pos_b": np.ascontiguousarray(inputs["pos_b"], f32),
        "tab1": np.ascontiguousarray(inputs["tab1"], f32),
        "b1": np.ascontiguousarray(inputs["b1"], f32),
        "tab2": np.ascontiguousarray(inputs["tab2"], f32),
        "b2": np.ascontiguousarray(inputs["b2"], f32),
        "collapse_token": np.ascontiguousarray(inputs["collapse_token"], f32),
        "collapse_weight": np.ascontiguousarray(inputs["collapse_weight"], f32),
        "region_w": np.ascontiguousarray(inputs["region_w"], f32),
        "coef64": coef,
    }
    for nm in COND_NAMES:
        common[nm] = np.ascontiguousarray(inputs[nm], f32)
        common[nm + "_idx32"] = np.ascontiguousarray(inputs[nm + "_idx"]).astype(np.int32)

    in_maps = [dict(common, oh1=_OH1[c], oh2=_OH2[c], ohg=_OHG[c]) for c in range(N_CORES)]

    trace = bool(int(os.environ.get("BASS_KERNEL_TRACE", "0")))
    last_exc = None
    for _attempt in range(3):
        try:
            res = run_bass_kernel_spmd(nc, in_maps, core_ids=list(range(N_CORES)),
                                       trace=trace)
            break
        except Exception as e:   # transient LoadExecutable failures seen on axon
            last_exc = e
    else:
        raise last_exc

    kernel.last_results = res
    z = np.empty((L, L, ZC), np.float32)
    for c in range(N_CORES):
        r0 = SHARD * c
        for si, (j0, j1) in enumerate(SEGS):
            z[r0:r0 + SHARD, j0:j1] = res.results[c][f"z_seg{si}"]
    s_out = res.results[0]["s_out"]
    return s_out, z
